# revision 2
# baseline (speedup 1.0000x reference)
"""DeepseekVL2 MoE gate: fp16 main GEMM + stacked fp8 DoubleRow correction.

Contract: kernel(**inputs) takes FULL unsharded inputs
  hidden_states [4, 4096, 7168] f32, weight [256, 7168] f32,
  e_score_correction_bias [256] f32
and returns (topk_idx [16384, 8] int32, topk_weight [16384, 8] f32).

Strategy:
  - Data parallel: 16384 tokens -> 2048 per core x 8 cores.
  - logits*1024 = xh16@wh16 + 2^-12 * DRsum, where DRsum accumulates one
    DoubleRow fp8 matmul per k-tile pairing plane0=(xl*4096 as e4m3,
    w*1024 as e4m3) and plane1=(x as e4m3, wl*4096*1024 as e4m3). The
    DR pass runs ~2 fp8 MACs/cell/cycle, so the correction costs ~0.5x
    the main pass instead of the 2 extra fp16 passes of the 3-pass
    scheme. HW-validated logit error ~6e-6 (vs 1e-4 plain fp16, which
    flips too many top-k ties to pass).
  - Routing per 128-token tile on-chip with DVE max8/max_index/
    match_replace ops (tie semantics match jax top_k).
"""

import numpy as np
import ml_dtypes

import concourse.bacc as bacc
import concourse.mybir as mybir
from concourse.bass_utils import run_bass_kernel_spmd
from concourse.tile import TileContext

F16 = mybir.dt.float16
F32 = mybir.dt.float32
F8 = mybir.dt.float8e4
U32 = mybir.dt.uint32
I32 = mybir.dt.int32
NPF8 = ml_dtypes.float8_e4m3

N_CORES = 8
T_FULL = 16384
T_CORE = T_FULL // N_CORES          # 2048
H = 7168
E = 256
KT = H // 128                        # 56 contraction tiles
N_TILES = T_CORE // 128              # 16 token tiles per core
N_GROUP = 8
GROUP_SIZE = E // N_GROUP            # 32
TOPK_GROUP = 4
TOP_K = 8
ROUTED_SCALING = 2.5
W_SCALE = 1024.0                     # keeps wl in fp16-normal range
CS = 4096.0                          # fp8 residual scale
NEG_BIG = -1.0e30
WCHUNK = 7


def _build_nc():
    nc = bacc.Bacc(
        "TRN2",
        target_bir_lowering=False,
        debug=False,
        num_devices=N_CORES,
    )

    # x pre-shuffled to [p, tile, k, t]; fp8 planes [p, tile, k, 2, t]
    xh_d = nc.dram_tensor("xh", [128, N_TILES, KT, 128], F16, kind="ExternalInput").ap()
    xc_d = nc.dram_tensor("xc", [128, N_TILES, KT, 2, 128], F8, kind="ExternalInput").ap()
    # w chunked [p, chunk, k, e]; fp8 planes [p, chunk, k, 2, e]
    wh_d = nc.dram_tensor("wh", [128, 8, WCHUNK, E], F16, kind="ExternalInput").ap()
    wc_d = nc.dram_tensor("wc", [128, 8, WCHUNK, 2, E], F8, kind="ExternalInput").ap()
    bias_d = nc.dram_tensor("biasb", [128, E], F32, kind="ExternalInput").ap()
    idx_d = nc.dram_tensor("out_idx", [T_CORE, TOP_K], I32, kind="ExternalOutput").ap()
    w_d = nc.dram_tensor("out_w", [T_CORE, TOP_K], F32, kind="ExternalOutput").ap()

    X = mybir.AxisListType.X
    Alu = mybir.AluOpType

    with TileContext(nc) as tc:
        with (
            tc.tile_pool(name="wpool", bufs=1) as wpool,
            tc.tile_pool(name="xpool", bufs=3) as xpool,
            tc.tile_pool(name="spool", bufs=2) as spool,
            tc.tile_pool(name="small", bufs=2) as small,
            tc.tile_pool(name="psum", bufs=3, space="PSUM") as psum_pool,
        ):
            bias_sb = wpool.tile([128, E], F32, tag="bias")
            whc = [
                wpool.tile([128, WCHUNK, E], F16, tag=f"whc{c}", name=f"whc{c}")
                for c in range(KT // WCHUNK)
            ]
            wcc = [
                wpool.tile([128, WCHUNK, 2, E], F8, tag=f"wcc{c}", name=f"wcc{c}")
                for c in range(KT // WCHUNK)
            ]

            def wh_k(k):
                return whc[k // WCHUNK][:, k % WCHUNK, :]

            def wc_k(k):
                return wcc[k // WCHUNK][:, k % WCHUNK, :, :]

            for tt in range(N_TILES):
                t0 = tt * 128
                xh_t = xpool.tile([128, KT, 128], F16, tag="xh")
                xc_t = xpool.tile([128, KT, 2, 128], F8, tag="xc")
                if tt == 0:
                    # Ramp: need-order is all of (wh, xh0) for the fp16 main
                    # pass, then all of (wc, xc0) for the DR correction.
                    # Interleave k-wise within each phase, alternating the
                    # two HWDGE queues so every matmul trails its data only
                    # slightly.
                    XC = KT // 4  # 14

                    def xhq(eng, j):
                        eng.dma_start(
                            xh_t[:, j * XC : (j + 1) * XC, :],
                            xh_d[:, 0, j * XC : (j + 1) * XC, :],
                        )

                    def xcq(eng, j):
                        eng.dma_start(
                            xc_t[:, j * XC : (j + 1) * XC, :, :],
                            xc_d[:, 0, j * XC : (j + 1) * XC, :, :],
                        )

                    # phase 1: wh chunks + xh0 quarters (k-ordered)
                    nc.sync.dma_start(whc[0][:], wh_d[:, 0, :, :])
                    nc.scalar.dma_start(whc[1][:], wh_d[:, 1, :, :])
                    xhq(nc.sync, 0)
                    nc.scalar.dma_start(whc[2][:], wh_d[:, 2, :, :])
                    nc.sync.dma_start(whc[3][:], wh_d[:, 3, :, :])
                    xhq(nc.scalar, 1)
                    nc.sync.dma_start(whc[4][:], wh_d[:, 4, :, :])
                    nc.scalar.dma_start(whc[5][:], wh_d[:, 5, :, :])
                    xhq(nc.sync, 2)
                    nc.scalar.dma_start(whc[6][:], wh_d[:, 6, :, :])
                    nc.sync.dma_start(whc[7][:], wh_d[:, 7, :, :])
                    xhq(nc.scalar, 3)
                    # phase 2: wc chunks + xc0 quarters (k-ordered)
                    nc.sync.dma_start(wcc[0][:], wc_d[:, 0, :, :, :])
                    nc.scalar.dma_start(wcc[1][:], wc_d[:, 1, :, :, :])
                    xcq(nc.sync, 0)
                    nc.scalar.dma_start(wcc[2][:], wc_d[:, 2, :, :, :])
                    nc.sync.dma_start(wcc[3][:], wc_d[:, 3, :, :, :])
                    xcq(nc.scalar, 1)
                    nc.sync.dma_start(wcc[4][:], wc_d[:, 4, :, :, :])
                    nc.scalar.dma_start(wcc[5][:], wc_d[:, 5, :, :, :])
                    xcq(nc.sync, 2)
                    nc.scalar.dma_start(wcc[6][:], wc_d[:, 6, :, :, :])
                    nc.sync.dma_start(wcc[7][:], wc_d[:, 7, :, :, :])
                    xcq(nc.scalar, 3)
                    nc.scalar.dma_start(bias_sb[:], bias_d)
                else:
                    # steady state: xh on sync, xc on scalar (equal bytes)
                    nc.sync.dma_start(xh_t[:], xh_d[:, tt, :, :])
                    nc.scalar.dma_start(xc_t[:], xc_d[:, tt, :, :, :])

                psa = psum_pool.tile([128, E], F32, tag="psa")
                for k in range(KT):
                    nc.tensor.matmul(
                        psa[:], xh_t[:, k, :], wh_k(k),
                        start=(k == 0), stop=(k == KT - 1), skip_group_check=True,
                    )
                psb = psum_pool.tile([128, E], F32, tag="psb")
                for k in range(KT):
                    nc.tensor.matmul(
                        psb[:], xc_t[:, k, :, :], wc_k(k),
                        start=(k == 0), stop=(k == KT - 1), skip_group_check=True,
                        perf_mode=mybir.MatmulPerfMode.DoubleRow,
                    )

                # logits*1024 = psa + psb/4096; sigmoid applies 1/1024
                t_sb = spool.tile([128, E], F32, tag="scratch", name="t_sb")
                nc.scalar.activation(
                    t_sb[:], psb[:], mybir.ActivationFunctionType.Copy,
                    scale=1.0 / CS,
                )
                lg = spool.tile([128, E], F32, tag="lg")
                nc.vector.tensor_add(lg[:], psa[:], t_sb[:])
                scores = spool.tile([128, E], F32, tag="scores")
                nc.scalar.activation(
                    scores[:], lg[:],
                    mybir.ActivationFunctionType.Sigmoid,
                    scale=1.0 / W_SCALE,
                )

                # scores_for_choice = scores + bias
                sfc = spool.tile([128, E], F32, tag="sfc")
                nc.vector.tensor_add(sfc[:], scores[:], bias_sb[:])

                # per-group top-2 sum
                sfc_g = sfc[:].rearrange("p (g e) -> p g e", g=N_GROUP)
                g1 = small.tile([128, N_GROUP], F32, tag="g1")
                nc.vector.reduce_max(g1[:], sfc_g, axis=X)
                sfc_mr = spool.tile([128, E], F32, tag="scratch", name="sfc_mr")
                nc.vector.match_replace(sfc_mr[:], g1[:], sfc[:], NEG_BIG)
                g2 = small.tile([128, N_GROUP], F32, tag="g2")
                nc.vector.reduce_max(
                    g2[:], sfc_mr[:].rearrange("p (g e) -> p g e", g=N_GROUP), axis=X
                )
                gs = small.tile([128, N_GROUP], F32, tag="gs")
                nc.vector.tensor_add(gs[:], g1[:], g2[:])

                # top-4 groups -> 0/1 mask
                gsrt = small.tile([128, 8], F32, tag="gsrt")
                nc.vector.max(out=gsrt[:], in_=gs[:])
                gmask = small.tile([128, N_GROUP], F32, tag="gmask")
                nc.vector.tensor_scalar(
                    gmask[:], gs[:], gsrt[:, TOPK_GROUP - 1 : TOPK_GROUP], None,
                    op0=Alu.is_ge,
                )

                # tmp = sfc * mask
                tmp = spool.tile([128, E], F32, tag="tmp")
                nc.vector.tensor_mul(
                    tmp[:].rearrange("p (g e) -> p g e", g=N_GROUP),
                    sfc_g,
                    gmask[:].unsqueeze(2).to_broadcast([128, N_GROUP, GROUP_SIZE]),
                )

                # ordered top-8 of tmp
                v8 = small.tile([128, 8], F32, tag="v8")
                nc.vector.max(out=v8[:], in_=tmp[:])
                i8 = small.tile([128, 8], U32, tag="i8")
                nc.vector.max_index(i8[:], v8[:], tmp[:])

                # pull raw sigmoid scores at selected positions
                tmp_mr = spool.tile([128, E], F32, tag="scratch", name="tmp_mr")
                nc.vector.match_replace(tmp_mr[:], v8[:], tmp[:], NEG_BIG)
                sel = spool.tile([128, E], F32, tag="sel")
                nc.vector.tensor_scalar(
                    sel[:], tmp_mr[:], NEG_BIG, None, op0=Alu.is_equal
                )
                scsel = spool.tile([128, E], F32, tag="scsel")
                nc.vector.tensor_mul(scsel[:], scores[:], sel[:])
                s8 = small.tile([128, 8], F32, tag="s8")
                nc.vector.max(out=s8[:], in_=scsel[:])
                s8i = small.tile([128, 8], U32, tag="s8i")
                nc.vector.max_index(s8i[:], s8[:], scsel[:])

                # idx output
                idx_out = small.tile([128, TOP_K], I32, tag="idx_out")
                nc.vector.tensor_copy(idx_out[:], i8[:])
                nc.sync.dma_start(idx_d[t0 : t0 + 128, :], idx_out[:])

                # re-pair scores: w8[k] = sum_j s8[j]*(s8i[j]==i8[k])
                e8 = small.tile([128, 8, 8], F32, tag="e8")
                nc.vector.tensor_tensor(
                    e8[:],
                    s8i[:].unsqueeze(1).to_broadcast([128, 8, 8]),
                    i8[:].unsqueeze(2).to_broadcast([128, 8, 8]),
                    op=Alu.is_equal,
                )
                w64 = small.tile([128, 8, 8], F32, tag="w64")
                nc.vector.tensor_mul(
                    w64[:], e8[:], s8[:].unsqueeze(1).to_broadcast([128, 8, 8])
                )
                w8 = small.tile([128, 8], F32, tag="w8")
                nc.vector.reduce_sum(w8[:], w64[:], axis=X)

                ds = small.tile([128, 1], F32, tag="ds")
                nc.vector.reduce_sum(ds[:], s8[:], axis=X)
                rcp = small.tile([128, 1], F32, tag="rcp")
                nc.vector.reciprocal(rcp[:], ds[:])
                w_out = small.tile([128, TOP_K], F32, tag="w_out")
                nc.vector.tensor_scalar(
                    w_out[:], w8[:], rcp[:, 0:1], ROUTED_SCALING,
                    op0=Alu.mult, op1=Alu.mult,
                )
                nc.sync.dma_start(w_d[t0 : t0 + 128, :], w_out[:])

    nc.compile()
    return nc


_NC_CACHE = None


def _get_nc():
    global _NC_CACHE
    if _NC_CACHE is None:
        _NC_CACHE = _build_nc()
    return _NC_CACHE


def _prep_inputs(hidden_states, weight, e_score_correction_bias):
    x = np.ascontiguousarray(hidden_states, dtype=np.float32).reshape(T_FULL, H)
    wT = np.ascontiguousarray(np.asarray(weight, dtype=np.float32).T)  # [H, E]
    wS = wT * W_SCALE
    wh = wS.astype(np.float16)
    wl = wS - wh.astype(np.float32)
    wh8 = wS.astype(NPF8)
    wl8 = (wl * CS).astype(NPF8)

    wh_dev = np.ascontiguousarray(
        wh.reshape(8, WCHUNK, 128, E).transpose(2, 0, 1, 3)
    )  # [p, c, k, e]
    wh8_l = wh8.reshape(8, WCHUNK, 128, E).transpose(2, 0, 1, 3)
    wl8_l = wl8.reshape(8, WCHUNK, 128, E).transpose(2, 0, 1, 3)
    wc_dev = np.ascontiguousarray(np.stack([wh8_l, wl8_l], axis=3))  # [p,c,k,2,e]

    bias_b = np.ascontiguousarray(
        np.broadcast_to(
            np.asarray(e_score_correction_bias, dtype=np.float32)[None, :], (128, E)
        )
    )
    in_maps = []
    for c in range(N_CORES):
        xc_f32 = x[c * T_CORE : (c + 1) * T_CORE]  # [Tc, H]
        xh = xc_f32.astype(np.float16)
        xl8 = ((xc_f32 - xh.astype(np.float32)) * CS).astype(NPF8)
        xh8 = xc_f32.astype(NPF8)
        # [p, tile, k, t]
        xh_dev = np.ascontiguousarray(
            xh.reshape(N_TILES, 128, KT, 128).transpose(3, 0, 2, 1)
        )
        xl8_l = xl8.reshape(N_TILES, 128, KT, 128).transpose(3, 0, 2, 1)
        xh8_l = xh8.reshape(N_TILES, 128, KT, 128).transpose(3, 0, 2, 1)
        xc_dev = np.ascontiguousarray(np.stack([xl8_l, xh8_l], axis=3))  # [p,tt,k,2,t]
        in_maps.append(
            {"xh": xh_dev, "xc": xc_dev, "wh": wh_dev, "wc": wc_dev, "biasb": bias_b}
        )
    return in_maps


def run(hidden_states, weight, e_score_correction_bias, trace=False, **spmd_kwargs):
    nc = _get_nc()
    in_maps = _prep_inputs(hidden_states, weight, e_score_correction_bias)
    res = run_bass_kernel_spmd(
        nc, in_maps, core_ids=list(range(N_CORES)), trace=trace, **spmd_kwargs
    )
    idx = np.concatenate([r["out_idx"] for r in res.results], axis=0)
    w = np.concatenate([r["out_w"] for r in res.results], axis=0)
    return (idx.astype(np.int32), w.astype(np.float32)), res


def kernel(hidden_states, weight, e_score_correction_bias):
    (idx, w), _ = run(hidden_states, weight, e_score_correction_bias, trace=False)
    return idx, w


# revision 3
# speedup vs baseline: 1.0219x; 1.0219x over previous
"""DeepseekVL2 MoE gate: fp16 main GEMM + stacked fp8 DoubleRow correction.

Contract: kernel(**inputs) takes FULL unsharded inputs
  hidden_states [4, 4096, 7168] f32, weight [256, 7168] f32,
  e_score_correction_bias [256] f32
and returns (topk_idx [16384, 8] int32, topk_weight [16384, 8] f32).

Strategy:
  - Data parallel: 16384 tokens -> 2048 per core x 8 cores.
  - logits*1024 = xh16@wh16 + 2^-12 * DRsum, where DRsum accumulates one
    DoubleRow fp8 matmul per k-tile pairing plane0=(xl*4096 as e4m3,
    w*1024 as e4m3) and plane1=(x as e4m3, wl*4096*1024 as e4m3). The
    DR pass runs ~2 fp8 MACs/cell/cycle, so the correction costs ~0.5x
    the main pass instead of the 2 extra fp16 passes of the 3-pass
    scheme. HW-validated logit error ~6e-6 (vs 1e-4 plain fp16, which
    flips too many top-k ties to pass).
  - Routing per 128-token tile on-chip with DVE max8/max_index/
    match_replace ops (tie semantics match jax top_k).
"""

import numpy as np
import ml_dtypes

import concourse.bacc as bacc
import concourse.mybir as mybir
from concourse.bass_utils import run_bass_kernel_spmd
from concourse.tile import TileContext

F16 = mybir.dt.float16
F32 = mybir.dt.float32
F8 = mybir.dt.float8e4
U32 = mybir.dt.uint32
I32 = mybir.dt.int32
NPF8 = ml_dtypes.float8_e4m3

N_CORES = 8
T_FULL = 16384
T_CORE = T_FULL // N_CORES          # 2048
H = 7168
E = 256
KT = H // 128                        # 56 contraction tiles
N_TILES = T_CORE // 128              # 16 token tiles per core
N_GROUP = 8
GROUP_SIZE = E // N_GROUP            # 32
TOPK_GROUP = 4
TOP_K = 8
ROUTED_SCALING = 2.5
W_SCALE = 1024.0                     # keeps wl in fp16-normal range
CS = 4096.0                          # fp8 residual scale
NEG_BIG = -1.0e30
WCHUNK = 7


def _build_nc():
    nc = bacc.Bacc(
        "TRN2",
        target_bir_lowering=False,
        debug=False,
        num_devices=N_CORES,
    )

    # x pre-shuffled to [p, tile, k, t]; fp8 planes [p, tile, k, 2, t]
    xh_d = nc.dram_tensor("xh", [128, N_TILES, KT, 128], F16, kind="ExternalInput").ap()
    xc_d = nc.dram_tensor("xc", [128, N_TILES, KT, 2, 128], F8, kind="ExternalInput").ap()
    # w chunked [p, chunk, k, e]; fp8 planes [p, chunk, k, 2, e]
    wh_d = nc.dram_tensor("wh", [128, 8, WCHUNK, E], F16, kind="ExternalInput").ap()
    wc_d = nc.dram_tensor("wc", [128, 8, WCHUNK, 2, E], F8, kind="ExternalInput").ap()
    bias_d = nc.dram_tensor("biasb", [128, E], F32, kind="ExternalInput").ap()
    idx_d = nc.dram_tensor("out_idx", [T_CORE, TOP_K], I32, kind="ExternalOutput").ap()
    w_d = nc.dram_tensor("out_w", [T_CORE, TOP_K], F32, kind="ExternalOutput").ap()

    X = mybir.AxisListType.X
    Alu = mybir.AluOpType

    with TileContext(nc) as tc:
        with (
            tc.tile_pool(name="wpool", bufs=1) as wpool,
            tc.tile_pool(name="xpool", bufs=4) as xpool,
            tc.tile_pool(name="spool", bufs=2) as spool,
            tc.tile_pool(name="small", bufs=2) as small,
            tc.tile_pool(name="psum", bufs=4, space="PSUM") as psum_pool,
        ):
            bias_sb = wpool.tile([128, E], F32, tag="bias")
            whc = [
                wpool.tile([128, WCHUNK, E], F16, tag=f"whc{c}", name=f"whc{c}")
                for c in range(KT // WCHUNK)
            ]
            wcc = [
                wpool.tile([128, WCHUNK, 2, E], F8, tag=f"wcc{c}", name=f"wcc{c}")
                for c in range(KT // WCHUNK)
            ]

            def wh_k(k):
                return whc[k // WCHUNK][:, k % WCHUNK, :]

            def wc_k(k):
                return wcc[k // WCHUNK][:, k % WCHUNK, :, :]

            for tt in range(N_TILES):
                t0 = tt * 128
                xh_t = xpool.tile([128, KT, 128], F16, tag="xh")
                xc_t = xpool.tile([128, KT, 2, 128], F8, tag="xc")
                if tt == 0:
                    # Ramp: need-order is all of (wh, xh0) for the fp16 main
                    # pass, then all of (wc, xc0) for the DR correction.
                    # Fine-grained ~230KB units interleaved k-wise across
                    # both HWDGE queues: per k-block j, wh chunk j (two
                    # halves) then xh eighth j, so matmul k trails its data
                    # by at most one unit.
                    qs = [nc.sync, nc.scalar]
                    qi = 0

                    def nq():
                        nonlocal qi
                        q = qs[qi & 1]
                        qi += 1
                        return q

                    for j in range(8):
                        nq().dma_start(whc[j][:, 0:4, :], wh_d[:, j, 0:4, :])
                        nq().dma_start(whc[j][:, 4:7, :], wh_d[:, j, 4:7, :])
                        nq().dma_start(
                            xh_t[:, j * 7 : (j + 1) * 7, :],
                            xh_d[:, 0, j * 7 : (j + 1) * 7, :],
                        )
                    for j in range(8):
                        nq().dma_start(wcc[j][:, 0:4, :, :], wc_d[:, j, 0:4, :, :])
                        nq().dma_start(wcc[j][:, 4:7, :, :], wc_d[:, j, 4:7, :, :])
                        nq().dma_start(
                            xc_t[:, j * 7 : (j + 1) * 7, :, :],
                            xc_d[:, 0, j * 7 : (j + 1) * 7, :, :],
                        )
                    nc.scalar.dma_start(bias_sb[:], bias_d)
                else:
                    # steady state: both queues carry the tile's xh halves
                    # first (main pass data lands by mid-cycle), then the
                    # xc halves for the DR pass.
                    XH2 = KT // 2
                    nc.sync.dma_start(
                        xh_t[:, 0:XH2, :], xh_d[:, tt, 0:XH2, :]
                    )
                    nc.scalar.dma_start(
                        xh_t[:, XH2:KT, :], xh_d[:, tt, XH2:KT, :]
                    )
                    nc.sync.dma_start(
                        xc_t[:, 0:XH2, :, :], xc_d[:, tt, 0:XH2, :, :]
                    )
                    nc.scalar.dma_start(
                        xc_t[:, XH2:KT, :, :], xc_d[:, tt, XH2:KT, :, :]
                    )

                psa = psum_pool.tile([128, E], F32, tag="psa")
                for k in range(KT):
                    nc.tensor.matmul(
                        psa[:], xh_t[:, k, :], wh_k(k),
                        start=(k == 0), stop=(k == KT - 1), skip_group_check=True,
                    )
                psb = psum_pool.tile([128, E], F32, tag="psb")
                for k in range(KT):
                    nc.tensor.matmul(
                        psb[:], xc_t[:, k, :, :], wc_k(k),
                        start=(k == 0), stop=(k == KT - 1), skip_group_check=True,
                        perf_mode=mybir.MatmulPerfMode.DoubleRow,
                    )

                # logits*1024 = psa + psb/4096; sigmoid applies 1/1024
                t_sb = spool.tile([128, E], F32, tag="scratch", name="t_sb")
                nc.scalar.activation(
                    t_sb[:], psb[:], mybir.ActivationFunctionType.Copy,
                    scale=1.0 / CS,
                )
                lg = spool.tile([128, E], F32, tag="lg")
                nc.vector.tensor_add(lg[:], psa[:], t_sb[:])
                scores = spool.tile([128, E], F32, tag="scores")
                nc.scalar.activation(
                    scores[:], lg[:],
                    mybir.ActivationFunctionType.Sigmoid,
                    scale=1.0 / W_SCALE,
                )

                # scores_for_choice = scores + bias
                sfc = spool.tile([128, E], F32, tag="sfc")
                nc.vector.tensor_add(sfc[:], scores[:], bias_sb[:])

                # per-group top-2 sum
                sfc_g = sfc[:].rearrange("p (g e) -> p g e", g=N_GROUP)
                g1 = small.tile([128, N_GROUP], F32, tag="g1")
                nc.vector.reduce_max(g1[:], sfc_g, axis=X)
                sfc_mr = spool.tile([128, E], F32, tag="scratch", name="sfc_mr")
                nc.vector.match_replace(sfc_mr[:], g1[:], sfc[:], NEG_BIG)
                g2 = small.tile([128, N_GROUP], F32, tag="g2")
                nc.vector.reduce_max(
                    g2[:], sfc_mr[:].rearrange("p (g e) -> p g e", g=N_GROUP), axis=X
                )
                gs = small.tile([128, N_GROUP], F32, tag="gs")
                nc.vector.tensor_add(gs[:], g1[:], g2[:])

                # top-4 groups -> 0/1 mask
                gsrt = small.tile([128, 8], F32, tag="gsrt")
                nc.vector.max(out=gsrt[:], in_=gs[:])
                gmask = small.tile([128, N_GROUP], F32, tag="gmask")
                nc.vector.tensor_scalar(
                    gmask[:], gs[:], gsrt[:, TOPK_GROUP - 1 : TOPK_GROUP], None,
                    op0=Alu.is_ge,
                )

                # tmp = sfc * mask
                tmp = spool.tile([128, E], F32, tag="tmp")
                nc.vector.tensor_mul(
                    tmp[:].rearrange("p (g e) -> p g e", g=N_GROUP),
                    sfc_g,
                    gmask[:].unsqueeze(2).to_broadcast([128, N_GROUP, GROUP_SIZE]),
                )

                # ordered top-8 of tmp
                v8 = small.tile([128, 8], F32, tag="v8")
                nc.vector.max(out=v8[:], in_=tmp[:])
                i8 = small.tile([128, 8], U32, tag="i8")
                nc.vector.max_index(i8[:], v8[:], tmp[:])

                # pull raw sigmoid scores at selected positions
                tmp_mr = spool.tile([128, E], F32, tag="scratch", name="tmp_mr")
                nc.vector.match_replace(tmp_mr[:], v8[:], tmp[:], NEG_BIG)
                sel = spool.tile([128, E], F32, tag="sel")
                nc.vector.tensor_scalar(
                    sel[:], tmp_mr[:], NEG_BIG, None, op0=Alu.is_equal
                )
                scsel = spool.tile([128, E], F32, tag="scsel")
                nc.vector.tensor_mul(scsel[:], scores[:], sel[:])
                s8 = small.tile([128, 8], F32, tag="s8")
                nc.vector.max(out=s8[:], in_=scsel[:])
                s8i = small.tile([128, 8], U32, tag="s8i")
                nc.vector.max_index(s8i[:], s8[:], scsel[:])

                # idx output
                idx_out = small.tile([128, TOP_K], I32, tag="idx_out")
                nc.vector.tensor_copy(idx_out[:], i8[:])
                nc.sync.dma_start(idx_d[t0 : t0 + 128, :], idx_out[:])

                # re-pair scores: w8[k] = sum_j s8[j]*(s8i[j]==i8[k])
                e8 = small.tile([128, 8, 8], F32, tag="e8")
                nc.vector.tensor_tensor(
                    e8[:],
                    s8i[:].unsqueeze(1).to_broadcast([128, 8, 8]),
                    i8[:].unsqueeze(2).to_broadcast([128, 8, 8]),
                    op=Alu.is_equal,
                )
                w64 = small.tile([128, 8, 8], F32, tag="w64")
                nc.vector.tensor_mul(
                    w64[:], e8[:], s8[:].unsqueeze(1).to_broadcast([128, 8, 8])
                )
                w8 = small.tile([128, 8], F32, tag="w8")
                nc.vector.reduce_sum(w8[:], w64[:], axis=X)

                ds = small.tile([128, 1], F32, tag="ds")
                nc.vector.reduce_sum(ds[:], s8[:], axis=X)
                rcp = small.tile([128, 1], F32, tag="rcp")
                nc.vector.reciprocal(rcp[:], ds[:])
                w_out = small.tile([128, TOP_K], F32, tag="w_out")
                nc.vector.tensor_scalar(
                    w_out[:], w8[:], rcp[:, 0:1], ROUTED_SCALING,
                    op0=Alu.mult, op1=Alu.mult,
                )
                nc.sync.dma_start(w_d[t0 : t0 + 128, :], w_out[:])

    nc.compile()
    return nc


_NC_CACHE = None


def _get_nc():
    global _NC_CACHE
    if _NC_CACHE is None:
        _NC_CACHE = _build_nc()
    return _NC_CACHE


def _prep_inputs(hidden_states, weight, e_score_correction_bias):
    x = np.ascontiguousarray(hidden_states, dtype=np.float32).reshape(T_FULL, H)
    wT = np.ascontiguousarray(np.asarray(weight, dtype=np.float32).T)  # [H, E]
    wS = wT * W_SCALE
    wh = wS.astype(np.float16)
    wl = wS - wh.astype(np.float32)
    wh8 = wS.astype(NPF8)
    wl8 = (wl * CS).astype(NPF8)

    wh_dev = np.ascontiguousarray(
        wh.reshape(8, WCHUNK, 128, E).transpose(2, 0, 1, 3)
    )  # [p, c, k, e]
    wh8_l = wh8.reshape(8, WCHUNK, 128, E).transpose(2, 0, 1, 3)
    wl8_l = wl8.reshape(8, WCHUNK, 128, E).transpose(2, 0, 1, 3)
    wc_dev = np.ascontiguousarray(np.stack([wh8_l, wl8_l], axis=3))  # [p,c,k,2,e]

    bias_b = np.ascontiguousarray(
        np.broadcast_to(
            np.asarray(e_score_correction_bias, dtype=np.float32)[None, :], (128, E)
        )
    )
    in_maps = []
    for c in range(N_CORES):
        xc_f32 = x[c * T_CORE : (c + 1) * T_CORE]  # [Tc, H]
        xh = xc_f32.astype(np.float16)
        xl8 = ((xc_f32 - xh.astype(np.float32)) * CS).astype(NPF8)
        xh8 = xc_f32.astype(NPF8)
        # [p, tile, k, t]
        xh_dev = np.ascontiguousarray(
            xh.reshape(N_TILES, 128, KT, 128).transpose(3, 0, 2, 1)
        )
        xl8_l = xl8.reshape(N_TILES, 128, KT, 128).transpose(3, 0, 2, 1)
        xh8_l = xh8.reshape(N_TILES, 128, KT, 128).transpose(3, 0, 2, 1)
        xc_dev = np.ascontiguousarray(np.stack([xl8_l, xh8_l], axis=3))  # [p,tt,k,2,t]
        in_maps.append(
            {"xh": xh_dev, "xc": xc_dev, "wh": wh_dev, "wc": wc_dev, "biasb": bias_b}
        )
    return in_maps


def run(hidden_states, weight, e_score_correction_bias, trace=False, **spmd_kwargs):
    nc = _get_nc()
    in_maps = _prep_inputs(hidden_states, weight, e_score_correction_bias)
    res = run_bass_kernel_spmd(
        nc, in_maps, core_ids=list(range(N_CORES)), trace=trace, **spmd_kwargs
    )
    idx = np.concatenate([r["out_idx"] for r in res.results], axis=0)
    w = np.concatenate([r["out_w"] for r in res.results], axis=0)
    return (idx.astype(np.int32), w.astype(np.float32)), res


def kernel(hidden_states, weight, e_score_correction_bias):
    (idx, w), _ = run(hidden_states, weight, e_score_correction_bias, trace=False)
    return idx, w


# revision 5
# speedup vs baseline: 1.0272x; 1.0052x over previous
"""DeepseekVL2 MoE gate: fp16 main GEMM + stacked fp8 DoubleRow correction.

Contract: kernel(**inputs) takes FULL unsharded inputs
  hidden_states [4, 4096, 7168] f32, weight [256, 7168] f32,
  e_score_correction_bias [256] f32
and returns (topk_idx [16384, 8] int32, topk_weight [16384, 8] f32).

Strategy:
  - Data parallel: 16384 tokens -> 2048 per core x 8 cores.
  - logits*1024 = xh16@wh16 + 2^-12 * DRsum, where DRsum accumulates one
    DoubleRow fp8 matmul per k-tile pairing plane0=(xl*4096 as e4m3,
    w*1024 as e4m3) and plane1=(x as e4m3, wl*4096*1024 as e4m3). The
    DR pass runs ~2 fp8 MACs/cell/cycle, so the correction costs ~0.5x
    the main pass instead of the 2 extra fp16 passes of the 3-pass
    scheme. HW-validated logit error ~6e-6 (vs 1e-4 plain fp16, which
    flips too many top-k ties to pass).
  - Routing per 128-token tile on-chip with DVE max8/max_index/
    match_replace ops (tie semantics match jax top_k).
  - ~80 dependency-free warm-up matmuls on a zeroed tile run during the
    ~9us DMA launch dead-time so the PE enters the real work at the
    K=8/8 HAM clock state instead of half rate.
"""

import numpy as np
import ml_dtypes

import concourse.bacc as bacc
import concourse.mybir as mybir
from concourse.bass_utils import run_bass_kernel_spmd
from concourse.tile import TileContext

F16 = mybir.dt.float16
F32 = mybir.dt.float32
F8 = mybir.dt.float8e4
U32 = mybir.dt.uint32
I32 = mybir.dt.int32
NPF8 = ml_dtypes.float8_e4m3

N_CORES = 8
T_FULL = 16384
T_CORE = T_FULL // N_CORES          # 2048
H = 7168
E = 256
KT = H // 128                        # 56 contraction tiles
N_TILES = T_CORE // 128              # 16 token tiles per core
N_GROUP = 8
GROUP_SIZE = E // N_GROUP            # 32
TOPK_GROUP = 4
TOP_K = 8
ROUTED_SCALING = 2.5
W_SCALE = 1024.0                     # keeps wl in fp16-normal range
CS = 4096.0                          # fp8 residual scale
NEG_BIG = -1.0e30
WCHUNK = 7


def _build_nc():
    nc = bacc.Bacc(
        "TRN2",
        target_bir_lowering=False,
        debug=False,
        num_devices=N_CORES,
    )

    # x pre-shuffled to [p, tile, k, t]; fp8 planes [p, tile, k, 2, t]
    xh_d = nc.dram_tensor("xh", [128, N_TILES, KT, 128], F16, kind="ExternalInput").ap()
    xc_d = nc.dram_tensor("xc", [128, N_TILES, KT, 2, 128], F8, kind="ExternalInput").ap()
    # w chunked [p, chunk, k, e]; fp8 planes [p, chunk, k, 2, e]
    wh_d = nc.dram_tensor("wh", [128, 8, WCHUNK, E], F16, kind="ExternalInput").ap()
    wc_d = nc.dram_tensor("wc", [128, 8, WCHUNK, 2, E], F8, kind="ExternalInput").ap()
    bias_d = nc.dram_tensor("biasb", [128, E], F32, kind="ExternalInput").ap()
    idx_d = nc.dram_tensor("out_idx", [T_CORE, TOP_K], I32, kind="ExternalOutput").ap()
    w_d = nc.dram_tensor("out_w", [T_CORE, TOP_K], F32, kind="ExternalOutput").ap()

    X = mybir.AxisListType.X
    Alu = mybir.AluOpType

    with TileContext(nc) as tc:
        with (
            tc.tile_pool(name="wpool", bufs=1) as wpool,
            tc.tile_pool(name="xpool", bufs=4) as xpool,
            tc.tile_pool(name="spool", bufs=2) as spool,
            tc.tile_pool(name="small", bufs=2) as small,
            tc.tile_pool(name="psum", bufs=3, space="PSUM") as psum_pool,
            tc.tile_pool(name="pfpool", bufs=1, space="PSUM") as pfpool,
        ):
            bias_sb = wpool.tile([128, E], F32, tag="bias")
            whc = [
                wpool.tile([128, WCHUNK, E], F16, tag=f"whc{c}", name=f"whc{c}")
                for c in range(KT // WCHUNK)
            ]
            wcc = [
                wpool.tile([128, WCHUNK, 2, E], F8, tag=f"wcc{c}", name=f"wcc{c}")
                for c in range(KT // WCHUNK)
            ]

            def wh_k(k):
                return whc[k // WCHUNK][:, k % WCHUNK, :]

            def wc_k(k):
                return wcc[k // WCHUNK][:, k % WCHUNK, :, :]

            # HAM pre-warm: dependency-free matmuls on a zeroed tile run
            # during the ~9us DMA launch dead-time, so the PE hits the
            # first real matmul already at the K=8/8 clock state.
            filler = wpool.tile([128, E], F16, tag="filler")
            nc.gpsimd.memset(filler[:], 0)
            pf = pfpool.tile([128, E], F32, tag="pf")
            for _ in range(80):
                nc.tensor.matmul(
                    pf[:], filler[:, 0:128], filler[:],
                    start=True, stop=True, skip_group_check=True,
                )

            for tt in range(N_TILES):
                t0 = tt * 128
                xh_t = xpool.tile([128, KT, 128], F16, tag="xh")
                xc_t = xpool.tile([128, KT, 2, 128], F8, tag="xc")
                if tt == 0:
                    # Ramp: need-order is all of (wh, xh0) for the fp16 main
                    # pass, then all of (wc, xc0) for the DR correction.
                    # Fine-grained ~230KB units interleaved k-wise across
                    # both HWDGE queues: per k-block j, wh chunk j (two
                    # halves) then xh eighth j, so matmul k trails its data
                    # by at most one unit.
                    qs = [nc.sync, nc.scalar]
                    qi = 0

                    def nq():
                        nonlocal qi
                        q = qs[qi & 1]
                        qi += 1
                        return q

                    nc.sync.dma_start(whc[0][:, 0:1, :], wh_d[:, 0, 0:1, :])
                    nc.scalar.dma_start(
                        xh_t[:, 0:1, :], xh_d[:, 0, 0:1, :]
                    )
                    for j in range(8):
                        if j == 0:
                            nq().dma_start(whc[0][:, 1:4, :], wh_d[:, 0, 1:4, :])
                            nq().dma_start(whc[0][:, 4:7, :], wh_d[:, 0, 4:7, :])
                            nq().dma_start(
                                xh_t[:, 1:7, :], xh_d[:, 0, 1:7, :]
                            )
                            continue
                        nq().dma_start(whc[j][:, 0:4, :], wh_d[:, j, 0:4, :])
                        nq().dma_start(whc[j][:, 4:7, :], wh_d[:, j, 4:7, :])
                        nq().dma_start(
                            xh_t[:, j * 7 : (j + 1) * 7, :],
                            xh_d[:, 0, j * 7 : (j + 1) * 7, :],
                        )
                    for j in range(8):
                        nq().dma_start(wcc[j][:, 0:4, :, :], wc_d[:, j, 0:4, :, :])
                        nq().dma_start(wcc[j][:, 4:7, :, :], wc_d[:, j, 4:7, :, :])
                        nq().dma_start(
                            xc_t[:, j * 7 : (j + 1) * 7, :, :],
                            xc_d[:, 0, j * 7 : (j + 1) * 7, :, :],
                        )
                    nc.scalar.dma_start(bias_sb[:], bias_d)
                else:
                    # steady state: both queues carry the tile's xh halves
                    # first (main pass data lands by mid-cycle), then the
                    # xc halves for the DR pass.
                    XH2 = KT // 2
                    nc.sync.dma_start(
                        xh_t[:, 0:XH2, :], xh_d[:, tt, 0:XH2, :]
                    )
                    nc.scalar.dma_start(
                        xh_t[:, XH2:KT, :], xh_d[:, tt, XH2:KT, :]
                    )
                    nc.sync.dma_start(
                        xc_t[:, 0:XH2, :, :], xc_d[:, tt, 0:XH2, :, :]
                    )
                    nc.scalar.dma_start(
                        xc_t[:, XH2:KT, :, :], xc_d[:, tt, XH2:KT, :, :]
                    )

                psa = psum_pool.tile([128, E], F32, tag="psa")
                for k in range(KT):
                    nc.tensor.matmul(
                        psa[:], xh_t[:, k, :], wh_k(k),
                        start=(k == 0), stop=(k == KT - 1), skip_group_check=True,
                    )
                psb = psum_pool.tile([128, E], F32, tag="psb")
                for k in range(KT):
                    nc.tensor.matmul(
                        psb[:], xc_t[:, k, :, :], wc_k(k),
                        start=(k == 0), stop=(k == KT - 1), skip_group_check=True,
                        perf_mode=mybir.MatmulPerfMode.DoubleRow,
                    )

                # logits*1024 = psa + psb/4096; sigmoid applies 1/1024
                t_sb = spool.tile([128, E], F32, tag="scratch", name="t_sb")
                nc.scalar.activation(
                    t_sb[:], psb[:], mybir.ActivationFunctionType.Copy,
                    scale=1.0 / CS,
                )
                lg = spool.tile([128, E], F32, tag="lg")
                nc.vector.tensor_add(lg[:], psa[:], t_sb[:])
                scores = spool.tile([128, E], F32, tag="scores")
                nc.scalar.activation(
                    scores[:], lg[:],
                    mybir.ActivationFunctionType.Sigmoid,
                    scale=1.0 / W_SCALE,
                )

                # scores_for_choice = scores + bias
                sfc = spool.tile([128, E], F32, tag="sfc")
                nc.vector.tensor_add(sfc[:], scores[:], bias_sb[:])

                # per-group top-2 sum
                sfc_g = sfc[:].rearrange("p (g e) -> p g e", g=N_GROUP)
                g1 = small.tile([128, N_GROUP], F32, tag="g1")
                nc.vector.reduce_max(g1[:], sfc_g, axis=X)
                sfc_mr = spool.tile([128, E], F32, tag="scratch", name="sfc_mr")
                nc.vector.match_replace(sfc_mr[:], g1[:], sfc[:], NEG_BIG)
                g2 = small.tile([128, N_GROUP], F32, tag="g2")
                nc.vector.reduce_max(
                    g2[:], sfc_mr[:].rearrange("p (g e) -> p g e", g=N_GROUP), axis=X
                )
                gs = small.tile([128, N_GROUP], F32, tag="gs")
                nc.vector.tensor_add(gs[:], g1[:], g2[:])

                # top-4 groups -> 0/1 mask
                gsrt = small.tile([128, 8], F32, tag="gsrt")
                nc.vector.max(out=gsrt[:], in_=gs[:])
                gmask = small.tile([128, N_GROUP], F32, tag="gmask")
                nc.vector.tensor_scalar(
                    gmask[:], gs[:], gsrt[:, TOPK_GROUP - 1 : TOPK_GROUP], None,
                    op0=Alu.is_ge,
                )

                # tmp = sfc * mask
                tmp = spool.tile([128, E], F32, tag="tmp")
                nc.vector.tensor_mul(
                    tmp[:].rearrange("p (g e) -> p g e", g=N_GROUP),
                    sfc_g,
                    gmask[:].unsqueeze(2).to_broadcast([128, N_GROUP, GROUP_SIZE]),
                )

                # ordered top-8 of tmp
                v8 = small.tile([128, 8], F32, tag="v8")
                nc.vector.max(out=v8[:], in_=tmp[:])
                i8 = small.tile([128, 8], U32, tag="i8")
                nc.vector.max_index(i8[:], v8[:], tmp[:])

                # pull raw sigmoid scores at selected positions
                tmp_mr = spool.tile([128, E], F32, tag="scratch", name="tmp_mr")
                nc.vector.match_replace(tmp_mr[:], v8[:], tmp[:], NEG_BIG)
                sel = spool.tile([128, E], F32, tag="sel")
                nc.vector.tensor_scalar(
                    sel[:], tmp_mr[:], NEG_BIG, None, op0=Alu.is_equal
                )
                scsel = spool.tile([128, E], F32, tag="scsel")
                nc.vector.tensor_mul(scsel[:], scores[:], sel[:])
                s8 = small.tile([128, 8], F32, tag="s8")
                nc.vector.max(out=s8[:], in_=scsel[:])
                s8i = small.tile([128, 8], U32, tag="s8i")
                nc.vector.max_index(s8i[:], s8[:], scsel[:])

                # idx output
                idx_out = small.tile([128, TOP_K], I32, tag="idx_out")
                nc.vector.tensor_copy(idx_out[:], i8[:])
                nc.sync.dma_start(idx_d[t0 : t0 + 128, :], idx_out[:])

                # re-pair scores: w8[k] = sum_j s8[j]*(s8i[j]==i8[k])
                e8 = small.tile([128, 8, 8], F32, tag="e8")
                nc.vector.tensor_tensor(
                    e8[:],
                    s8i[:].unsqueeze(1).to_broadcast([128, 8, 8]),
                    i8[:].unsqueeze(2).to_broadcast([128, 8, 8]),
                    op=Alu.is_equal,
                )
                w64 = small.tile([128, 8, 8], F32, tag="w64")
                nc.vector.tensor_mul(
                    w64[:], e8[:], s8[:].unsqueeze(1).to_broadcast([128, 8, 8])
                )
                w8 = small.tile([128, 8], F32, tag="w8")
                nc.vector.reduce_sum(w8[:], w64[:], axis=X)

                ds = small.tile([128, 1], F32, tag="ds")
                nc.vector.reduce_sum(ds[:], s8[:], axis=X)
                rcp = small.tile([128, 1], F32, tag="rcp")
                nc.vector.reciprocal(rcp[:], ds[:])
                w_out = small.tile([128, TOP_K], F32, tag="w_out")
                nc.vector.tensor_scalar(
                    w_out[:], w8[:], rcp[:, 0:1], ROUTED_SCALING,
                    op0=Alu.mult, op1=Alu.mult,
                )
                nc.sync.dma_start(w_d[t0 : t0 + 128, :], w_out[:])

    nc.compile()
    return nc


_NC_CACHE = None


def _get_nc():
    global _NC_CACHE
    if _NC_CACHE is None:
        _NC_CACHE = _build_nc()
    return _NC_CACHE


def _prep_inputs(hidden_states, weight, e_score_correction_bias):
    x = np.ascontiguousarray(hidden_states, dtype=np.float32).reshape(T_FULL, H)
    wT = np.ascontiguousarray(np.asarray(weight, dtype=np.float32).T)  # [H, E]
    wS = wT * W_SCALE
    wh = wS.astype(np.float16)
    wl = wS - wh.astype(np.float32)
    wh8 = wS.astype(NPF8)
    wl8 = (wl * CS).astype(NPF8)

    wh_dev = np.ascontiguousarray(
        wh.reshape(8, WCHUNK, 128, E).transpose(2, 0, 1, 3)
    )  # [p, c, k, e]
    wh8_l = wh8.reshape(8, WCHUNK, 128, E).transpose(2, 0, 1, 3)
    wl8_l = wl8.reshape(8, WCHUNK, 128, E).transpose(2, 0, 1, 3)
    wc_dev = np.ascontiguousarray(np.stack([wh8_l, wl8_l], axis=3))  # [p,c,k,2,e]

    bias_b = np.ascontiguousarray(
        np.broadcast_to(
            np.asarray(e_score_correction_bias, dtype=np.float32)[None, :], (128, E)
        )
    )
    in_maps = []
    for c in range(N_CORES):
        xc_f32 = x[c * T_CORE : (c + 1) * T_CORE]  # [Tc, H]
        xh = xc_f32.astype(np.float16)
        xl8 = ((xc_f32 - xh.astype(np.float32)) * CS).astype(NPF8)
        xh8 = xc_f32.astype(NPF8)
        # [p, tile, k, t]
        xh_dev = np.ascontiguousarray(
            xh.reshape(N_TILES, 128, KT, 128).transpose(3, 0, 2, 1)
        )
        xl8_l = xl8.reshape(N_TILES, 128, KT, 128).transpose(3, 0, 2, 1)
        xh8_l = xh8.reshape(N_TILES, 128, KT, 128).transpose(3, 0, 2, 1)
        xc_dev = np.ascontiguousarray(np.stack([xl8_l, xh8_l], axis=3))  # [p,tt,k,2,t]
        in_maps.append(
            {"xh": xh_dev, "xc": xc_dev, "wh": wh_dev, "wc": wc_dev, "biasb": bias_b}
        )
    return in_maps


def run(hidden_states, weight, e_score_correction_bias, trace=False, **spmd_kwargs):
    nc = _get_nc()
    in_maps = _prep_inputs(hidden_states, weight, e_score_correction_bias)
    res = run_bass_kernel_spmd(
        nc, in_maps, core_ids=list(range(N_CORES)), trace=trace, **spmd_kwargs
    )
    idx = np.concatenate([r["out_idx"] for r in res.results], axis=0)
    w = np.concatenate([r["out_w"] for r in res.results], axis=0)
    return (idx.astype(np.int32), w.astype(np.float32)), res


def kernel(hidden_states, weight, e_score_correction_bias):
    (idx, w), _ = run(hidden_states, weight, e_score_correction_bias, trace=False)
    return idx, w
